# revision 16
# baseline (speedup 1.0000x reference)
"""Trainium2 Bass kernel for BatteryMoEFlattenIntraCycleMoELayer.

Computation (reference):
    gates = renorm(top2(softmax(logits) * mask))          # [B, E]
    x = cycle_curve_data.reshape(B, L, 900)
    out[b] = sum_e gates[b,e] * (x[b] @ W[e] + b[e])      # -> bf16 [B, L, 512]

Strategy (bf16, gate-prescaled x, 7 full K-chunks + row-tiled remainder):
  - Host computes gates/top-2; x is augmented with a bias row (K=901)
    and prescaled by each selected gate -> two copies per sample.
    K = 7 full chunks of 128 + a 5-row remainder (feats 896..899 +
    bias).  Remainder rows are replicated at partitions {0,32,64,96}
    so remainder matmuls for 4 samples run CONCURRENTLY in the 4 PE
    row-groups (tile_position), cutting the old zero-padded chunk-7
    cost ~4x.  (W offsets must stay tile-direct with min_val=0:
    shifting them by a column base broke row-groups 1/3 on HW.)
  - Expert slots are permuted so the 4 most-used experts occupy the
    lo W columns; every core's first 16 samples are chosen (globally,
    via sample permutation) to route only to lo slots, enforced via
    RuntimeValue bounds, so the phase-1 critical stream is
    widx + 8 x [xph1[k] (512 KB) + w_lo[k] (512 KB)] on the Sync
    HWDGE ring -- chunk arrival ~3.6 us vs 3.45 us PE consumption.
    W hi columns stream behind phase 1.
  - Head: ~12 junk matmuls on a zeroed tile warm the PE (HAM 8/8)
    while the first DMAs land.  Phase-2 per-sample x rides the Scalar
    (ACT) HWDGE ring behind a FIFO gate (a dummy read of w_sb[7])
    that releases only after the phase-1 stream has fully landed;
    phase-2 x tiles also share a 9-buffer pool with the phase-1
    tiles, so their DMA self-throttles to the compute rate.
  - Phase 1: samples 0-7 k-outer (8 PSUM banks).  Phase 2: quads of
    sample-major full chunks + one concurrent remainder batch pair;
    combines on DVE (tensor_scalar_add, psum f32 -> sbuf bf16), y on
    the Sync ring; in the last quad the final sample combines/stores
    first to shorten the tail.
  - Shard B across 8 cores (64 samples each, host-permuted; output
    inverse-permuted).
"""

import os
import sys

for _p in ("/opt/trn_rl_repo", "/root/.axon_site/_ro/trn_rl_repo"):
    if os.path.isdir(_p) and _p not in sys.path:
        sys.path.insert(0, _p)

import numpy as np
import ml_dtypes

import concourse.bass as bass
import concourse.mybir as mybir
import concourse.tile as tile
from concourse import bacc
from concourse.bass_utils import run_bass_kernel_spmd
from concourse.bass_values import RuntimeValue

B, L, CURVE_LEN = 512, 128, 300
FEAT = 3 * CURVE_LEN          # 900
FEAT_AUG = FEAT + 1           # 901 (bias row)
NKF = 7                       # full 128-row K chunks (rows 0..895)
REM = FEAT_AUG - NKF * 128    # 5 remainder rows (896..899 + bias)
D_MODEL = 512
NUM_EXPERTS = 8
TOP_K = 2
EPS = 1e-9
N_CORES = 8
S = B // N_CORES              # 64 samples per core
NP1 = 8                       # phase-1 k-outer group size (PSUM banks)
NLOW = 16                     # per-core samples guaranteed lo-routed
NLO_E = 4                     # experts in the lo slot group
XCOLS = 2 * NP1 * L           # 2048: phase-1 x columns per chunk
WBASE = 0                     # widx offsets are direct W columns
LO_COLS = NLO_E * D_MODEL     # 2048
WCOLS = NUM_EXPERTS * D_MODEL # 4096
LOWMAX = (NLO_E - 1) * D_MODEL
WMAX = (NUM_EXPERTS - 1) * D_MODEL
NJUNK = 12                    # PE-warmup matmuls

BF16 = ml_dtypes.bfloat16

_CACHE = {}


def _build_nc(full_lowmax=False):
    """Build the SPMD Bass program (routing carried as data)."""
    nc = bacc.Bacc(trn_type="TRN2")
    f32 = mybir.dt.float32
    bf16 = mybir.dt.bfloat16
    i32 = mybir.dt.int32

    lowmax = WMAX if full_lowmax else LOWMAX

    # phase-1 x, k-major: col = (s*2 + j)*128 + l for samples 0..7
    xph1_h = nc.declare_dram_parameter("xph1", [8, 128, XCOLS], bf16,
                                       isOutput=False)
    # w per k-chunk: [k, part, slot*512]; chunk 7 = remainder rows
    # replicated at partitions {0,32,64,96}
    w_h = nc.declare_dram_parameter("w", [8, 128, WCOLS], bf16,
                                    isOutput=False)
    # phase-2 x, sample-major: col = (j*8 + k)*128 + l  (k=7 -> remainder)
    x2_h = nc.declare_dram_parameter("x2", [S - NP1, 128, 2 * 8 * L], bf16,
                                     isOutput=False)
    widx_h = nc.declare_dram_parameter("widx", [1, 2 * S], i32, isOutput=False)
    y_h = nc.declare_dram_parameter("y", [S, L, D_MODEL], bf16, isOutput=True)

    with tile.TileContext(nc) as tc:
        with (
            tc.tile_pool(name="cpool", bufs=1) as cpool,
            tc.tile_pool(name="xpool", bufs=9) as xpool,
            tc.tile_pool(name="opool", bufs=10) as opool,
            tc.tile_pool(name="pspool", bufs=8, space="PSUM") as pspool,
        ):
            # ---- head: widx + junk-warmup ----
            widx_sb = cpool.tile([1, 2 * S], i32)
            nc.sync.dma_start(out=widx_sb[:, :], in_=widx_h[:, :])

            junk = xpool.tile([128, 2 * 8 * L], bf16, tag="x", name="junk")
            nc.vector.memset(junk[:, 0:640], 0.0)
            ps_junk = pspool.tile([128, D_MODEL], f32, tag="ps",
                                  name="ps_junk")
            for _ in range(NJUNK):
                nc.tensor.matmul(ps_junk[:, :], junk[:, 0:128],
                                 junk[:, 128:640], start=True, stop=True)

            # ---- critical stream on the Sync ring: xph1[k] + w_lo[k] ----
            xph1_sb = []
            w_sb = []
            for k in range(8):
                xt = xpool.tile([128, XCOLS], bf16, tag="x",
                                name=f"xph1_{k}")
                nc.sync.dma_start(out=xt[:, :], in_=xph1_h[k, :, :])
                xph1_sb.append(xt)
                wt = cpool.tile([128, WCOLS], bf16, name=f"w_sb_{k}")
                nc.sync.dma_start(out=wt[:, 0:LO_COLS],
                                  in_=w_h[k, :, 0:LO_COLS])
                w_sb.append(wt)

            # ring of PE registers for per-sample W-slot offsets
            NRING = 16
            wregs = [nc.tensor.alloc_register(f"widx_reg{i}")
                     for i in range(NRING)]

            rv_of = {}

            def load_rv8(s0, maxv):
                # 8 registers <- widx for samples s0..s0+3 in one load;
                # consecutive batches land in alternating ring halves
                regs = [wregs[(2 * s0 + j) % NRING] for j in range(8)]
                nc.tensor.reg_load(regs, widx_sb[0:1, 2 * s0: 2 * s0 + 8])
                for j in range(4):
                    if s0 + j < S:
                        rv_of[s0 + j] = (
                            RuntimeValue(val=regs[2 * j], min_val=WBASE,
                                         max_val=maxv),
                            RuntimeValue(val=regs[2 * j + 1], min_val=WBASE,
                                         max_val=maxv),
                        )

            def load_rv16(s0, maxv):
                load_rv8(s0, maxv)
                load_rv8(s0 + 4, maxv)

            def mm_full(ps, lhs, k, rv, start):
                nc.tensor.matmul(
                    ps[:, :], lhs,
                    w_sb[k][:, bass.ds(rv, D_MODEL)],
                    start=start, stop=False,
                )

            def mm_rem(ps, lhs_tile, col, i, rv, stop):
                # remainder rows at partitions 32i..32i+4; 4 distinct
                # row-groups run concurrently on the PE
                nc.tensor.matmul(
                    ps[:, :],
                    lhs_tile[32 * i: 32 * i + REM, col: col + L],
                    w_sb[7][32 * i: 32 * i + REM, bass.ds(rv, D_MODEL)],
                    start=False, stop=stop,
                    tile_position=(32 * i, 0),
                )

            def combine(s, ps):
                o_sb = opool.tile([128, D_MODEL], bf16, tag="o", name=f"o_{s}")
                nc.vector.tensor_scalar_add(o_sb[:, :], ps[:, :], 0.0)
                nc.sync.dma_start(out=y_h[s, :, :], in_=o_sb[:, :])

            # ---- phase 1: samples 0..7, k-outer ----
            load_rv16(0, lowmax)
            ps1 = {s: pspool.tile([128, D_MODEL], f32, tag="ps",
                                  name=f"ps_{s}") for s in range(NP1)}
            for k in range(NKF):
                for s in range(NP1):
                    rvA, rvB = rv_of[s]
                    mm_full(ps1[s],
                            xph1_sb[k][:, (2 * s) * L:(2 * s + 1) * L],
                            k, rvA, start=(k == 0))
                    mm_full(ps1[s],
                            xph1_sb[k][:, (2 * s + 1) * L:(2 * s + 2) * L],
                            k, rvB, start=False)

            # w hi columns stream behind the phase-1 critical path
            for k in range(8):
                nc.sync.dma_start(out=w_sb[k][:, LO_COLS:WCOLS],
                                  in_=w_h[k, :, LO_COLS:WCOLS])

            # phase-1 remainder: concurrent 4-slot batches, then combine
            for g in range(2):
                for j in range(2):
                    for i in range(4):
                        s = 4 * g + i
                        rv = rv_of[s][j]
                        mm_rem(ps1[s], xph1_sb[7], (2 * s + j) * L, i, rv,
                               stop=(j == 1))
                for i in range(4):
                    s = 4 * g + i
                    combine(s, ps1[s])

            # ---- phase 2: samples 8..63, quad-major ----
            # FIFO gate on the ACT ring: the first x2 trigger sits behind
            # this read of h_sb[7], so phase-2 x DMA can't steal HBM
            # bandwidth from the phase-1 critical stream.
            gate_sb = cpool.tile([1, 16], bf16, name="gate_sb")
            nc.scalar.copy(gate_sb[0:1, :], w_sb[7][0:1, 0:16])

            x2_sb = {}
            psq = {}
            for s in range(NP1, S):
                xt = xpool.tile([128, 2 * 8 * L], bf16, tag="x",
                                name=f"x2_{s}")
                nc.scalar.dma_start(out=xt[:, :], in_=x2_h[s - NP1, :, :])
                x2_sb[s] = xt

                if s % 8 == 0:
                    load_rv16(s, lowmax if s + 8 <= NLOW else WMAX)
                rvA, rvB = rv_of[s]

                ps = pspool.tile([128, D_MODEL], f32, tag="ps",
                                 name=f"ps2_{s}")
                psq[s] = ps
                for k in range(NKF):
                    mm_full(ps, xt[:, k * L:(k + 1) * L], k, rvA,
                            start=(k == 0))
                    mm_full(ps, xt[:, (8 + k) * L:(9 + k) * L], k, rvB,
                            start=False)

                if s % 4 == 3:
                    q0 = s - 3
                    for j in range(2):
                        for i in range(4):
                            s2 = q0 + i
                            rv = rv_of[s2][j]
                            mm_rem(psq[s2], x2_sb[s2], (j * 8 + 7) * L, i,
                                   rv, stop=(j == 1))
                    # last quad: combine/store the final sample FIRST so
                    # its y DMA isn't serialized behind 3 other combines
                    idxs = range(3, -1, -1) if s == S - 1 else range(4)
                    for i in idxs:
                        s2 = q0 + i
                        combine(s2, psq[s2])
                        del x2_sb[s2], psq[s2]

    nc.finalize()
    return nc


def _gates_np(logits, moe_masks):
    """Mirror reference _gates in numpy (fp32)."""
    lg = logits.astype(np.float32)
    m = lg.max(axis=1, keepdims=True)
    e = np.exp(lg - m)
    g = e / e.sum(axis=1, keepdims=True)
    g = g * (moe_masks == 1).astype(np.float32)
    # top-2, ties -> lower index first (matches jax.lax.top_k)
    top_idx = np.argsort(-g, axis=1, kind="stable")[:, :TOP_K]
    rows = np.arange(g.shape[0])[:, None]
    gsel = g[rows, top_idx]                                  # [B, 2]
    gsel = gsel / (gsel.sum(axis=1, keepdims=True) + EPS)
    return gsel.astype(np.float32), top_idx.astype(np.int32)


def _routing_plan(gsel, top_idx):
    """Pick the lo expert set, slot permutation, and per-core sample order."""
    zero = gsel.sum(axis=1) == 0
    pair_mask = np.zeros(B, np.int64)
    for j in range(TOP_K):
        pair_mask |= np.int64(1) << top_idx[:, j].astype(np.int64)
    pair_mask[zero] = 0  # zero-gate rows can claim any slots
    import itertools
    best, best_cnt = None, -1
    for sub in itertools.combinations(range(NUM_EXPERTS), NLO_E):
        msk = np.int64(sum(1 << e for e in sub))
        cnt = int(((pair_mask & ~msk) == 0).sum())
        if cnt > best_cnt:
            best, best_cnt = sub, cnt
    lo_set = list(best)
    hi_set = [e for e in range(NUM_EXPERTS) if e not in lo_set]
    perm = np.empty(NUM_EXPERTS, np.int64)     # expert -> slot
    for slot, e in enumerate(lo_set + hi_set):
        perm[e] = slot

    slot_idx = perm[top_idx]                   # [B, 2]
    slot_idx[zero] = [0, 1]
    low = slot_idx.max(axis=1) < NLO_E

    low_ids = np.where(low)[0]
    high_ids = np.where(~low)[0]
    full_low = len(low_ids) >= NLOW * N_CORES
    order = np.empty((N_CORES, S), np.int64)
    if full_low:
        rest = np.concatenate([low_ids[NLOW * N_CORES:], high_ids])
        for c in range(N_CORES):
            order[c, :NLOW] = low_ids[c * NLOW:(c + 1) * NLOW]
            order[c, NLOW:] = rest[c * (S - NLOW):(c + 1) * (S - NLOW)]
    else:  # fallback: no lo guarantee; program must use full_lowmax
        allb = np.arange(B)
        for c in range(N_CORES):
            order[c] = allb[c * S:(c + 1) * S]
    return perm, slot_idx, order, full_low


def _prep_inputs(cycle_curve_data, logits, moe_masks, W, b):
    gsel, top_idx = _gates_np(logits, moe_masks)
    perm, slot_idx, order, full_low = _routing_plan(gsel, top_idx)

    xf = cycle_curve_data.reshape(B, L, FEAT).astype(np.float32, copy=False)
    # gate-prescaled augmented x: xs[b, j, l, f], f in [0, 901)
    xs = np.empty((B, 2, L, FEAT_AUG), np.float32)
    xs[:, 0, :, :FEAT] = xf * gsel[:, 0, None, None]
    xs[:, 1, :, :FEAT] = xf * gsel[:, 1, None, None]
    xs[:, 0, :, FEAT] = gsel[:, 0, None]
    xs[:, 1, :, FEAT] = gsel[:, 1, None]

    # full[b, p, j, k, l]; k<7 from rows k*128+p, k=7 remainder replicas
    full = np.zeros((B, 128, 2, 8, L), BF16)
    main = xs[:, :, :, :NKF * 128].reshape(B, 2, L, NKF, 128)
    full[:, :, :, :NKF, :] = main.transpose(0, 4, 1, 3, 2).astype(BF16)
    remT = xs[:, :, :, NKF * 128:].transpose(0, 3, 1, 2).astype(BF16)
    for i in range(4):
        full[:, 32 * i:32 * i + REM, :, NKF, :] = remT

    # W with permuted expert slots
    w_aug = np.zeros((NUM_EXPERTS, FEAT_AUG, D_MODEL), np.float32)
    w_aug[perm, :FEAT, :] = W.astype(np.float32)
    w_aug[perm, FEAT, :] = b.astype(np.float32)
    wt = np.zeros((8, 128, NUM_EXPERTS * D_MODEL), BF16)
    wm = w_aug[:, :NKF * 128, :].reshape(NUM_EXPERTS, NKF, 128, D_MODEL)
    wt[:NKF] = (wm.transpose(1, 2, 0, 3)
                .reshape(NKF, 128, NUM_EXPERTS * D_MODEL).astype(BF16))
    wr = w_aug[:, NKF * 128:, :].transpose(1, 0, 2).reshape(
        REM, NUM_EXPERTS * D_MODEL)
    for i in range(4):
        wt[NKF, 32 * i:32 * i + REM, :] = wr.astype(BF16)

    in_maps = []
    for c in range(N_CORES):
        ids = order[c]
        sel = full[ids]                              # [S, 128, 2, 8, L]
        xph1 = np.ascontiguousarray(
            sel[:NP1].transpose(3, 1, 0, 2, 4)       # [k, p, s, j, l]
        ).reshape(8, 128, XCOLS)
        x2 = np.ascontiguousarray(sel[NP1:]).reshape(S - NP1, 128, 2 * 8 * L)
        widx = (slot_idx[ids].reshape(1, 2 * S) * D_MODEL).astype(np.int32)
        in_maps.append({"xph1": xph1, "x2": x2, "w": wt, "widx": widx})
    return in_maps, order, full_low


def kernel(cycle_curve_data, logits, moe_masks, W, b):
    in_maps, order, full_low = _prep_inputs(
        cycle_curve_data, logits, moe_masks, W, b)

    key = "nc" if full_low else "nc_full"
    if key not in _CACHE:
        _CACHE[key] = _build_nc(full_lowmax=not full_low)
    nc = _CACHE[key]

    trace = bool(int(os.environ.get("KERNEL_PROFILE", "0")))
    res = run_bass_kernel_spmd(
        nc, in_maps, core_ids=list(range(N_CORES)), trace=trace
    )
    _CACHE["last_results"] = res

    out = np.empty((B, L, D_MODEL), ml_dtypes.bfloat16)
    for c in range(N_CORES):
        out[order[c]] = res.results[c]["y"]
    return out


# revision 18
# speedup vs baseline: 1.1729x; 1.1729x over previous
"""Trainium2 Bass kernel for BatteryMoEFlattenIntraCycleMoELayer.

Computation (reference):
    gates = renorm(top2(softmax(logits) * mask))          # [B, E]
    x = cycle_curve_data.reshape(B, L, 900)
    out[b] = sum_e gates[b,e] * (x[b] @ W[e] + b[e])      # -> bf16 [B, L, 512]

Strategy (bf16, gate-prescaled x, 7 full K-chunks + row-tiled remainder):
  - Host computes gates/top-2; x is augmented with a bias row (K=901)
    and prescaled by each selected gate -> two copies per sample.
    K = 7 full chunks of 128 + a 5-row remainder (feats 896..899 +
    bias).  Remainder rows are replicated at partitions {0,32,64,96}
    so remainder matmuls for 4 samples run CONCURRENTLY in the 4 PE
    row-groups (tile_position), cutting the old zero-padded chunk-7
    cost ~4x.  (W offsets must stay tile-direct with min_val=0:
    shifting them by a column base broke row-groups 1/3 on HW.)
  - Expert slots are permuted so the 4 most-used experts occupy the
    lo W columns; every core's first 16 samples are chosen (globally,
    via sample permutation) to route only to lo slots, enforced via
    RuntimeValue bounds, so the phase-1 critical stream is
    widx + 8 x [xph1[k] (512 KB) + w_lo[k] (512 KB)] on the Sync
    HWDGE ring -- chunk arrival ~3.6 us vs 3.45 us PE consumption.
    W hi columns stream behind phase 1.
  - Head: ~12 junk matmuls on a zeroed tile warm the PE (HAM 8/8)
    while the first DMAs land.  Phase-2 per-sample x rides the Scalar
    (ACT) HWDGE ring behind a FIFO gate (a dummy read of w_sb[7])
    that releases only after the phase-1 stream has fully landed;
    phase-2 x tiles also share a 9-buffer pool with the phase-1
    tiles, so their DMA self-throttles to the compute rate.
  - Phase 1: samples 0-7 k-outer (8 PSUM banks).  Phase 2: quads of
    sample-major full chunks + one concurrent remainder batch pair;
    combines on DVE (tensor_scalar_add, psum f32 -> sbuf bf16), y on
    the Sync ring; in the last quad the final sample combines/stores
    first to shorten the tail.
  - Shard B across 8 cores (64 samples each, host-permuted; output
    inverse-permuted).
"""

import os
import sys

for _p in ("/opt/trn_rl_repo", "/root/.axon_site/_ro/trn_rl_repo"):
    if os.path.isdir(_p) and _p not in sys.path:
        sys.path.insert(0, _p)

import numpy as np
import ml_dtypes

import concourse.bass as bass
import concourse.mybir as mybir
import concourse.tile as tile
from concourse import bacc
from concourse.bass_utils import run_bass_kernel_spmd
from concourse.bass_values import RuntimeValue

B, L, CURVE_LEN = 512, 128, 300
FEAT = 3 * CURVE_LEN          # 900
FEAT_AUG = FEAT + 1           # 901 (bias row)
NKF = 7                       # full 128-row K chunks (rows 0..895)
REM = FEAT_AUG - NKF * 128    # 5 remainder rows (896..899 + bias)
D_MODEL = 512
NUM_EXPERTS = 8
TOP_K = 2
EPS = 1e-9
N_CORES = 8
S = B // N_CORES              # 64 samples per core
NP1 = 8                       # phase-1 k-outer group size (PSUM banks)
NLOW = 16                     # per-core samples guaranteed lo-routed
NLO_E = 4                     # experts in the lo slot group
XCOLS = 2 * NP1 * L           # 2048: phase-1 x columns per chunk
WBASE = 0                     # widx offsets are direct W columns
LO_COLS = NLO_E * D_MODEL     # 2048
WCOLS = NUM_EXPERTS * D_MODEL # 4096
LOWMAX = (NLO_E - 1) * D_MODEL
WMAX = (NUM_EXPERTS - 1) * D_MODEL
NJUNK = 12                    # PE-warmup matmuls

BF16 = ml_dtypes.bfloat16

_CACHE = {}


def _build_nc(full_lowmax=False):
    """Build the SPMD Bass program (routing carried as data)."""
    nc = bacc.Bacc(trn_type="TRN2")
    f32 = mybir.dt.float32
    bf16 = mybir.dt.bfloat16
    i32 = mybir.dt.int32

    lowmax = WMAX if full_lowmax else LOWMAX

    # phase-1 x, k-major: col = (s*2 + j)*128 + l for samples 0..7
    xph1_h = nc.declare_dram_parameter("xph1", [8, 128, XCOLS], bf16,
                                       isOutput=False)
    # w per k-chunk: [k, part, slot*512]; chunk 7 = remainder rows
    # replicated at partitions {0,32,64,96}
    w_h = nc.declare_dram_parameter("w", [8, 128, WCOLS], bf16,
                                    isOutput=False)
    # phase-2 x, sample-major: col = (j*8 + k)*128 + l  (k=7 -> remainder)
    x2_h = nc.declare_dram_parameter("x2", [S - NP1, 128, 2 * 8 * L], bf16,
                                     isOutput=False)
    widx_h = nc.declare_dram_parameter("widx", [1, 2 * S], i32, isOutput=False)
    y_h = nc.declare_dram_parameter("y", [S, L, D_MODEL], bf16, isOutput=True)

    with tile.TileContext(nc) as tc:
        with (
            tc.tile_pool(name="cpool", bufs=1) as cpool,
            tc.tile_pool(name="xpool", bufs=9) as xpool,
            tc.tile_pool(name="opool", bufs=10) as opool,
            tc.tile_pool(name="pspool", bufs=8, space="PSUM") as pspool,
        ):
            # ---- head: widx + junk-warmup ----
            widx_sb = cpool.tile([1, 2 * S], i32)
            nc.sync.dma_start(out=widx_sb[:, :], in_=widx_h[:, :])

            junk = xpool.tile([128, 2 * 8 * L], bf16, tag="x", name="junk")
            nc.vector.memset(junk[:, 0:640], 0.0)
            ps_junk = pspool.tile([128, D_MODEL], f32, tag="ps",
                                  name="ps_junk")
            for _ in range(NJUNK):
                nc.tensor.matmul(ps_junk[:, :], junk[:, 0:128],
                                 junk[:, 128:640], start=True, stop=True)

            # ---- critical stream on the Sync ring: xph1[k] + w_lo[k] ----
            xph1_sb = []
            w_sb = []
            for k in range(8):
                xt = xpool.tile([128, XCOLS], bf16, tag="x",
                                name=f"xph1_{k}")
                nc.scalar.dma_start(out=xt[:, :], in_=xph1_h[k, :, :])
                xph1_sb.append(xt)
                wt = cpool.tile([128, WCOLS], bf16, name=f"w_sb_{k}")
                nc.sync.dma_start(out=wt[:, 0:LO_COLS],
                                  in_=w_h[k, :, 0:LO_COLS])
                w_sb.append(wt)

            # ring of PE registers for per-sample W-slot offsets
            NRING = 16
            wregs = [nc.tensor.alloc_register(f"widx_reg{i}")
                     for i in range(NRING)]

            rv_of = {}

            def load_rv8(s0, maxv):
                # 8 registers <- widx for samples s0..s0+3 in one load;
                # consecutive batches land in alternating ring halves
                regs = [wregs[(2 * s0 + j) % NRING] for j in range(8)]
                nc.tensor.reg_load(regs, widx_sb[0:1, 2 * s0: 2 * s0 + 8])
                for j in range(4):
                    if s0 + j < S:
                        rv_of[s0 + j] = (
                            RuntimeValue(val=regs[2 * j], min_val=WBASE,
                                         max_val=maxv),
                            RuntimeValue(val=regs[2 * j + 1], min_val=WBASE,
                                         max_val=maxv),
                        )

            def load_rv16(s0, maxv):
                load_rv8(s0, maxv)
                load_rv8(s0 + 4, maxv)

            def mm_full(ps, lhs, k, rv, start):
                nc.tensor.matmul(
                    ps[:, :], lhs,
                    w_sb[k][:, bass.ds(rv, D_MODEL)],
                    start=start, stop=False,
                )

            def mm_rem(ps, lhs_tile, col, i, rv, stop):
                # remainder rows at partitions 32i..32i+4; 4 distinct
                # row-groups run concurrently on the PE
                nc.tensor.matmul(
                    ps[:, :],
                    lhs_tile[32 * i: 32 * i + REM, col: col + L],
                    w_sb[7][32 * i: 32 * i + REM, bass.ds(rv, D_MODEL)],
                    start=False, stop=stop,
                    tile_position=(32 * i, 0),
                )

            def combine(s, ps):
                o_sb = opool.tile([128, D_MODEL], bf16, tag="o", name=f"o_{s}")
                nc.vector.tensor_scalar_add(o_sb[:, :], ps[:, :], 0.0)
                nc.sync.dma_start(out=y_h[s, :, :], in_=o_sb[:, :])

            # ---- phase 1: samples 0..7, k-outer ----
            load_rv16(0, lowmax)
            ps1 = {s: pspool.tile([128, D_MODEL], f32, tag="ps",
                                  name=f"ps_{s}") for s in range(NP1)}
            for k in range(NKF):
                for s in range(NP1):
                    rvA, rvB = rv_of[s]
                    mm_full(ps1[s],
                            xph1_sb[k][:, (2 * s) * L:(2 * s + 1) * L],
                            k, rvA, start=(k == 0))
                    mm_full(ps1[s],
                            xph1_sb[k][:, (2 * s + 1) * L:(2 * s + 2) * L],
                            k, rvB, start=False)

            # w hi columns stream behind the phase-1 critical path
            for k in range(8):
                nc.sync.dma_start(out=w_sb[k][:, LO_COLS:WCOLS],
                                  in_=w_h[k, :, LO_COLS:WCOLS])

            # phase-1 remainder: concurrent 4-slot batches, then combine
            for g in range(2):
                for j in range(2):
                    for i in range(4):
                        s = 4 * g + i
                        rv = rv_of[s][j]
                        mm_rem(ps1[s], xph1_sb[7], (2 * s + j) * L, i, rv,
                               stop=(j == 1))
                for i in range(4):
                    s = 4 * g + i
                    combine(s, ps1[s])

            # ---- phase 2: samples 8..63, quad-major ----
            # FIFO gate on the ACT ring: the first x2 trigger sits behind
            # this read of h_sb[7], so phase-2 x DMA can't steal HBM
            # bandwidth from the phase-1 critical stream.
            gate_sb = cpool.tile([1, 16], bf16, name="gate_sb")
            nc.scalar.copy(gate_sb[0:1, :], w_sb[7][0:1, 0:16])

            # rem batching in hexads (6 samples) + a final duo: fewer
            # full<->rem PE config transitions per sample than quads,
            # while group + successor PSUM banks stay within 8.
            group_end = {NP1 + 6 * h + 5 for h in range(9)} | {S - 1}
            group_of = []
            g0 = NP1
            for s in range(NP1, S):
                group_of.append(g0)
                if s in group_end:
                    g0 = s + 1

            x2_sb = {}
            psq = {}
            for s in range(NP1, S):
                xt = xpool.tile([128, 2 * 8 * L], bf16, tag="x",
                                name=f"x2_{s}")
                nc.scalar.dma_start(out=xt[:, :], in_=x2_h[s - NP1, :, :])
                x2_sb[s] = xt

                if s % 4 == 0:
                    load_rv8(s, lowmax if s + 4 <= NLOW else WMAX)
                rvA, rvB = rv_of[s]

                ps = pspool.tile([128, D_MODEL], f32, tag="ps",
                                 name=f"ps2_{s}")
                psq[s] = ps
                for k in range(NKF):
                    mm_full(ps, xt[:, k * L:(k + 1) * L], k, rvA,
                            start=(k == 0))
                    mm_full(ps, xt[:, (8 + k) * L:(9 + k) * L], k, rvB,
                            start=False)

                if s in group_end:
                    q0 = group_of[s - NP1]
                    n = s - q0 + 1
                    # (sample, j) slots in batches of 4 across row-groups;
                    # each bank's j=1 rem MM is pc-later than its j=0 one,
                    # so stop rides the j=1 MM.
                    slots = [(q0 + i, j) for j in range(2) for i in range(n)]
                    bs = 4 if n >= 4 else n  # a batch must not repeat a bank
                    for bi in range(0, 2 * n, bs):
                        for rg, (s2, j) in enumerate(slots[bi:bi + bs]):
                            rv = rv_of[s2][j]
                            mm_rem(psq[s2], x2_sb[s2], (j * 8 + 7) * L, rg,
                                   rv, stop=(j == 1))
                    # last group: combine/store the final sample FIRST so
                    # its y DMA isn't serialized behind other combines
                    idxs = range(n - 1, -1, -1) if s == S - 1 else range(n)
                    for i in idxs:
                        s2 = q0 + i
                        combine(s2, psq[s2])
                        del x2_sb[s2], psq[s2]

    nc.finalize()
    return nc


def _gates_np(logits, moe_masks):
    """Mirror reference _gates in numpy (fp32)."""
    lg = logits.astype(np.float32)
    m = lg.max(axis=1, keepdims=True)
    e = np.exp(lg - m)
    g = e / e.sum(axis=1, keepdims=True)
    g = g * (moe_masks == 1).astype(np.float32)
    # top-2, ties -> lower index first (matches jax.lax.top_k)
    top_idx = np.argsort(-g, axis=1, kind="stable")[:, :TOP_K]
    rows = np.arange(g.shape[0])[:, None]
    gsel = g[rows, top_idx]                                  # [B, 2]
    gsel = gsel / (gsel.sum(axis=1, keepdims=True) + EPS)
    return gsel.astype(np.float32), top_idx.astype(np.int32)


def _routing_plan(gsel, top_idx):
    """Pick the lo expert set, slot permutation, and per-core sample order."""
    zero = gsel.sum(axis=1) == 0
    pair_mask = np.zeros(B, np.int64)
    for j in range(TOP_K):
        pair_mask |= np.int64(1) << top_idx[:, j].astype(np.int64)
    pair_mask[zero] = 0  # zero-gate rows can claim any slots
    import itertools
    best, best_cnt = None, -1
    for sub in itertools.combinations(range(NUM_EXPERTS), NLO_E):
        msk = np.int64(sum(1 << e for e in sub))
        cnt = int(((pair_mask & ~msk) == 0).sum())
        if cnt > best_cnt:
            best, best_cnt = sub, cnt
    lo_set = list(best)
    hi_set = [e for e in range(NUM_EXPERTS) if e not in lo_set]
    perm = np.empty(NUM_EXPERTS, np.int64)     # expert -> slot
    for slot, e in enumerate(lo_set + hi_set):
        perm[e] = slot

    slot_idx = perm[top_idx]                   # [B, 2]
    slot_idx[zero] = [0, 1]
    low = slot_idx.max(axis=1) < NLO_E

    low_ids = np.where(low)[0]
    high_ids = np.where(~low)[0]
    full_low = len(low_ids) >= NLOW * N_CORES
    order = np.empty((N_CORES, S), np.int64)
    if full_low:
        rest = np.concatenate([low_ids[NLOW * N_CORES:], high_ids])
        for c in range(N_CORES):
            order[c, :NLOW] = low_ids[c * NLOW:(c + 1) * NLOW]
            order[c, NLOW:] = rest[c * (S - NLOW):(c + 1) * (S - NLOW)]
    else:  # fallback: no lo guarantee; program must use full_lowmax
        allb = np.arange(B)
        for c in range(N_CORES):
            order[c] = allb[c * S:(c + 1) * S]
    return perm, slot_idx, order, full_low


def _prep_inputs(cycle_curve_data, logits, moe_masks, W, b):
    gsel, top_idx = _gates_np(logits, moe_masks)
    perm, slot_idx, order, full_low = _routing_plan(gsel, top_idx)

    xf = cycle_curve_data.reshape(B, L, FEAT).astype(np.float32, copy=False)
    # gate-prescaled augmented x: xs[b, j, l, f], f in [0, 901)
    xs = np.empty((B, 2, L, FEAT_AUG), np.float32)
    xs[:, 0, :, :FEAT] = xf * gsel[:, 0, None, None]
    xs[:, 1, :, :FEAT] = xf * gsel[:, 1, None, None]
    xs[:, 0, :, FEAT] = gsel[:, 0, None]
    xs[:, 1, :, FEAT] = gsel[:, 1, None]

    # full[b, p, j, k, l]; k<7 from rows k*128+p, k=7 remainder replicas
    full = np.zeros((B, 128, 2, 8, L), BF16)
    main = xs[:, :, :, :NKF * 128].reshape(B, 2, L, NKF, 128)
    full[:, :, :, :NKF, :] = main.transpose(0, 4, 1, 3, 2).astype(BF16)
    remT = xs[:, :, :, NKF * 128:].transpose(0, 3, 1, 2).astype(BF16)
    for i in range(4):
        full[:, 32 * i:32 * i + REM, :, NKF, :] = remT

    # W with permuted expert slots
    w_aug = np.zeros((NUM_EXPERTS, FEAT_AUG, D_MODEL), np.float32)
    w_aug[perm, :FEAT, :] = W.astype(np.float32)
    w_aug[perm, FEAT, :] = b.astype(np.float32)
    wt = np.zeros((8, 128, NUM_EXPERTS * D_MODEL), BF16)
    wm = w_aug[:, :NKF * 128, :].reshape(NUM_EXPERTS, NKF, 128, D_MODEL)
    wt[:NKF] = (wm.transpose(1, 2, 0, 3)
                .reshape(NKF, 128, NUM_EXPERTS * D_MODEL).astype(BF16))
    wr = w_aug[:, NKF * 128:, :].transpose(1, 0, 2).reshape(
        REM, NUM_EXPERTS * D_MODEL)
    for i in range(4):
        wt[NKF, 32 * i:32 * i + REM, :] = wr.astype(BF16)

    in_maps = []
    for c in range(N_CORES):
        ids = order[c]
        sel = full[ids]                              # [S, 128, 2, 8, L]
        xph1 = np.ascontiguousarray(
            sel[:NP1].transpose(3, 1, 0, 2, 4)       # [k, p, s, j, l]
        ).reshape(8, 128, XCOLS)
        x2 = np.ascontiguousarray(sel[NP1:]).reshape(S - NP1, 128, 2 * 8 * L)
        widx = (slot_idx[ids].reshape(1, 2 * S) * D_MODEL).astype(np.int32)
        in_maps.append({"xph1": xph1, "x2": x2, "w": wt, "widx": widx})
    return in_maps, order, full_low


def kernel(cycle_curve_data, logits, moe_masks, W, b):
    in_maps, order, full_low = _prep_inputs(
        cycle_curve_data, logits, moe_masks, W, b)

    key = "nc" if full_low else "nc_full"
    if key not in _CACHE:
        _CACHE[key] = _build_nc(full_lowmax=not full_low)
    nc = _CACHE[key]

    trace = bool(int(os.environ.get("KERNEL_PROFILE", "0")))
    res = run_bass_kernel_spmd(
        nc, in_maps, core_ids=list(range(N_CORES)), trace=trace
    )
    _CACHE["last_results"] = res

    out = np.empty((B, L, D_MODEL), ml_dtypes.bfloat16)
    for c in range(N_CORES):
        out[order[c]] = res.results[c]["y"]
    return out


# revision 19
# speedup vs baseline: 1.1792x; 1.0053x over previous
"""Trainium2 Bass kernel for BatteryMoEFlattenIntraCycleMoELayer.

Computation (reference):
    gates = renorm(top2(softmax(logits) * mask))          # [B, E]
    x = cycle_curve_data.reshape(B, L, 900)
    out[b] = sum_e gates[b,e] * (x[b] @ W[e] + b[e])      # -> bf16 [B, L, 512]

Strategy (bf16, gate-prescaled x, 7 full K-chunks + row-tiled remainder):
  - Host computes gates/top-2; x is augmented with a bias row (K=901)
    and prescaled by each selected gate -> two copies per sample.
    K = 7 full chunks of 128 + a 5-row remainder (feats 896..899 +
    bias).  Remainder rows are replicated at partitions {0,32,64,96}
    so remainder matmuls for 4 samples run CONCURRENTLY in the 4 PE
    row-groups (tile_position), cutting the old zero-padded chunk-7
    cost ~4x.  (W offsets must stay tile-direct with min_val=0:
    shifting them by a column base broke row-groups 1/3 on HW.)
  - Expert slots are permuted so the 4 most-used experts occupy the
    lo W columns; every core's first 16 samples are chosen (globally,
    via sample permutation) to route only to lo slots, enforced via
    RuntimeValue bounds, so the phase-1 critical stream is
    widx + 8 x [xph1[k] (512 KB) + w_lo[k] (512 KB)] on the Sync
    HWDGE ring -- chunk arrival ~3.6 us vs 3.45 us PE consumption.
    W hi columns stream behind phase 1.
  - Head: ~12 junk matmuls on a zeroed tile warm the PE (HAM 8/8)
    while the first DMAs land.  Phase-2 per-sample x rides the Scalar
    (ACT) HWDGE ring behind a FIFO gate (a dummy read of w_sb[7])
    that releases only after the phase-1 stream has fully landed;
    phase-2 x tiles also share a 9-buffer pool with the phase-1
    tiles, so their DMA self-throttles to the compute rate.
  - Phase 1: samples 0-7 k-outer (8 PSUM banks).  Phase 2: quads of
    sample-major full chunks + one concurrent remainder batch pair;
    combines on DVE (tensor_scalar_add, psum f32 -> sbuf bf16), y on
    the Sync ring; in the last quad the final sample combines/stores
    first to shorten the tail.
  - Shard B across 8 cores (64 samples each, host-permuted; output
    inverse-permuted).
"""

import os
import sys

for _p in ("/opt/trn_rl_repo", "/root/.axon_site/_ro/trn_rl_repo"):
    if os.path.isdir(_p) and _p not in sys.path:
        sys.path.insert(0, _p)

import numpy as np
import ml_dtypes

import concourse.bass as bass
import concourse.mybir as mybir
import concourse.tile as tile
from concourse import bacc
from concourse.bass_utils import run_bass_kernel_spmd
from concourse.bass_values import RuntimeValue

B, L, CURVE_LEN = 512, 128, 300
FEAT = 3 * CURVE_LEN          # 900
FEAT_AUG = FEAT + 1           # 901 (bias row)
NKF = 7                       # full 128-row K chunks (rows 0..895)
REM = FEAT_AUG - NKF * 128    # 5 remainder rows (896..899 + bias)
D_MODEL = 512
NUM_EXPERTS = 8
TOP_K = 2
EPS = 1e-9
N_CORES = 8
S = B // N_CORES              # 64 samples per core
NP1 = 8                       # phase-1 k-outer group size (PSUM banks)
NLOW = 16                     # per-core samples guaranteed lo-routed
NLO_E = 4                     # experts in the lo slot group
XCOLS = 2 * NP1 * L           # 2048: phase-1 x columns per chunk
WBASE = 0                     # widx offsets are direct W columns
LO_COLS = NLO_E * D_MODEL     # 2048
WCOLS = NUM_EXPERTS * D_MODEL # 4096
LOWMAX = (NLO_E - 1) * D_MODEL
WMAX = (NUM_EXPERTS - 1) * D_MODEL
NJUNK = 12                    # PE-warmup matmuls

BF16 = ml_dtypes.bfloat16

_CACHE = {}


def _build_nc(full_lowmax=False):
    """Build the SPMD Bass program (routing carried as data)."""
    nc = bacc.Bacc(trn_type="TRN2")
    f32 = mybir.dt.float32
    bf16 = mybir.dt.bfloat16
    i32 = mybir.dt.int32

    lowmax = WMAX if full_lowmax else LOWMAX

    # phase-1 x, k-major: col = (s*2 + j)*128 + l for samples 0..7
    xph1_h = nc.declare_dram_parameter("xph1", [8, 128, XCOLS], bf16,
                                       isOutput=False)
    # w per k-chunk: [k, part, slot*512]; chunk 7 = remainder rows
    # replicated at partitions {0,32,64,96}
    w_h = nc.declare_dram_parameter("w", [8, 128, WCOLS], bf16,
                                    isOutput=False)
    # phase-2 x, sample-major: col = (j*8 + k)*128 + l  (k=7 -> remainder)
    x2_h = nc.declare_dram_parameter("x2", [S - NP1, 128, 2 * 8 * L], bf16,
                                     isOutput=False)
    widx_h = nc.declare_dram_parameter("widx", [1, 2 * S], i32, isOutput=False)
    y_h = nc.declare_dram_parameter("y", [S, L, D_MODEL], bf16, isOutput=True)

    with tile.TileContext(nc) as tc:
        with (
            tc.tile_pool(name="cpool", bufs=1) as cpool,
            tc.tile_pool(name="xpool", bufs=9) as xpool,
            tc.tile_pool(name="opool", bufs=10) as opool,
            tc.tile_pool(name="pspool", bufs=8, space="PSUM") as pspool,
        ):
            # ---- head: widx + junk-warmup ----
            widx_sb = cpool.tile([1, 2 * S], i32)
            nc.sync.dma_start(out=widx_sb[:, :], in_=widx_h[:, :])

            junk = xpool.tile([128, 2 * 8 * L], bf16, tag="x", name="junk")
            nc.vector.memset(junk[:, 0:640], 0.0)
            ps_junk = pspool.tile([128, D_MODEL], f32, tag="ps",
                                  name="ps_junk")
            for _ in range(NJUNK):
                nc.tensor.matmul(ps_junk[:, :], junk[:, 0:128],
                                 junk[:, 128:640], start=True, stop=True)

            # ---- critical stream on the Sync ring: xph1[k] + w_lo[k] ----
            xph1_sb = []
            w_sb = []
            for k in range(8):
                xt = xpool.tile([128, XCOLS], bf16, tag="x",
                                name=f"xph1_{k}")
                nc.sync.dma_start(out=xt[:, :], in_=xph1_h[k, :, :])
                xph1_sb.append(xt)
                wt = cpool.tile([128, WCOLS], bf16, name=f"w_sb_{k}")
                nc.sync.dma_start(out=wt[:, 0:LO_COLS],
                                  in_=w_h[k, :, 0:LO_COLS])
                w_sb.append(wt)

            # ring of PE registers for per-sample W-slot offsets
            NRING = 16
            wregs = [nc.tensor.alloc_register(f"widx_reg{i}")
                     for i in range(NRING)]

            rv_of = {}

            def load_rv8(s0, maxv):
                # 8 registers <- widx for samples s0..s0+3 in one load;
                # consecutive batches land in alternating ring halves
                regs = [wregs[(2 * s0 + j) % NRING] for j in range(8)]
                nc.tensor.reg_load(regs, widx_sb[0:1, 2 * s0: 2 * s0 + 8])
                for j in range(4):
                    if s0 + j < S:
                        rv_of[s0 + j] = (
                            RuntimeValue(val=regs[2 * j], min_val=WBASE,
                                         max_val=maxv),
                            RuntimeValue(val=regs[2 * j + 1], min_val=WBASE,
                                         max_val=maxv),
                        )

            def load_rv16(s0, maxv):
                load_rv8(s0, maxv)
                load_rv8(s0 + 4, maxv)

            def mm_full(ps, lhs, k, rv, start):
                nc.tensor.matmul(
                    ps[:, :], lhs,
                    w_sb[k][:, bass.ds(rv, D_MODEL)],
                    start=start, stop=False,
                )

            def mm_rem(ps, lhs_tile, col, i, rv, stop):
                # remainder rows at partitions 32i..32i+4; 4 distinct
                # row-groups run concurrently on the PE
                nc.tensor.matmul(
                    ps[:, :],
                    lhs_tile[32 * i: 32 * i + REM, col: col + L],
                    w_sb[7][32 * i: 32 * i + REM, bass.ds(rv, D_MODEL)],
                    start=False, stop=stop,
                    tile_position=(32 * i, 0),
                )

            def combine(s, ps):
                o_sb = opool.tile([128, D_MODEL], bf16, tag="o", name=f"o_{s}")
                nc.vector.tensor_scalar_add(o_sb[:, :], ps[:, :], 0.0)
                nc.sync.dma_start(out=y_h[s, :, :], in_=o_sb[:, :])

            # ---- phase 1: samples 0..7, k-outer ----
            load_rv16(0, lowmax)
            ps1 = {s: pspool.tile([128, D_MODEL], f32, tag="ps",
                                  name=f"ps_{s}") for s in range(NP1)}
            for k in range(NKF):
                for s in range(NP1):
                    rvA, rvB = rv_of[s]
                    mm_full(ps1[s],
                            xph1_sb[k][:, (2 * s) * L:(2 * s + 1) * L],
                            k, rvA, start=(k == 0))
                    mm_full(ps1[s],
                            xph1_sb[k][:, (2 * s + 1) * L:(2 * s + 2) * L],
                            k, rvB, start=False)

            # w hi columns stream behind the phase-1 critical path
            for k in range(8):
                nc.sync.dma_start(out=w_sb[k][:, LO_COLS:WCOLS],
                                  in_=w_h[k, :, LO_COLS:WCOLS])

            # phase-1 remainder: concurrent 4-slot batches, then combine
            for g in range(2):
                for j in range(2):
                    for i in range(4):
                        s = 4 * g + i
                        rv = rv_of[s][j]
                        mm_rem(ps1[s], xph1_sb[7], (2 * s + j) * L, i, rv,
                               stop=(j == 1))
                for i in range(4):
                    s = 4 * g + i
                    combine(s, ps1[s])

            # ---- phase 2: samples 8..63, quad-major ----
            # FIFO gate on the ACT ring: the first x2 trigger sits behind
            # this read of h_sb[7], so phase-2 x DMA can't steal HBM
            # bandwidth from the phase-1 critical stream.
            gate_sb = cpool.tile([1, 16], bf16, name="gate_sb")
            nc.scalar.copy(gate_sb[0:1, :], w_sb[7][0:1, 0:16])

            # rem batching in hexads (6 samples) + a final duo: fewer
            # full<->rem PE config transitions per sample than quads,
            # while group + successor PSUM banks stay within 8.
            group_end = {NP1 + 6 * h + 5 for h in range(9)} | {S - 1}
            group_of = []
            g0 = NP1
            for s in range(NP1, S):
                group_of.append(g0)
                if s in group_end:
                    g0 = s + 1

            x2_sb = {}
            psq = {}
            for s in range(NP1, S):
                xt = xpool.tile([128, 2 * 8 * L], bf16, tag="x",
                                name=f"x2_{s}")
                nc.scalar.dma_start(out=xt[:, :], in_=x2_h[s - NP1, :, :])
                x2_sb[s] = xt

                if s % 4 == 0:
                    load_rv8(s, lowmax if s + 4 <= NLOW else WMAX)
                rvA, rvB = rv_of[s]

                ps = pspool.tile([128, D_MODEL], f32, tag="ps",
                                 name=f"ps2_{s}")
                psq[s] = ps
                for k in range(NKF):
                    mm_full(ps, xt[:, k * L:(k + 1) * L], k, rvA,
                            start=(k == 0))
                    mm_full(ps, xt[:, (8 + k) * L:(9 + k) * L], k, rvB,
                            start=False)

                if s in group_end:
                    q0 = group_of[s - NP1]
                    n = s - q0 + 1
                    # (sample, j) slots in batches of 4 across row-groups;
                    # each bank's j=1 rem MM is pc-later than its j=0 one,
                    # so stop rides the j=1 MM.
                    slots = [(q0 + i, j) for j in range(2) for i in range(n)]
                    bs = 4 if n >= 4 else n  # a batch must not repeat a bank
                    for bi in range(0, 2 * n, bs):
                        for rg, (s2, j) in enumerate(slots[bi:bi + bs]):
                            rv = rv_of[s2][j]
                            mm_rem(psq[s2], x2_sb[s2], (j * 8 + 7) * L, rg,
                                   rv, stop=(j == 1))
                    # last group: combine/store the final sample FIRST so
                    # its y DMA isn't serialized behind other combines
                    idxs = range(n - 1, -1, -1) if s == S - 1 else range(n)
                    for i in idxs:
                        s2 = q0 + i
                        combine(s2, psq[s2])
                        del x2_sb[s2], psq[s2]

    nc.finalize()
    return nc


def _gates_np(logits, moe_masks):
    """Mirror reference _gates in numpy (fp32)."""
    lg = logits.astype(np.float32)
    m = lg.max(axis=1, keepdims=True)
    e = np.exp(lg - m)
    g = e / e.sum(axis=1, keepdims=True)
    g = g * (moe_masks == 1).astype(np.float32)
    # top-2, ties -> lower index first (matches jax.lax.top_k)
    top_idx = np.argsort(-g, axis=1, kind="stable")[:, :TOP_K]
    rows = np.arange(g.shape[0])[:, None]
    gsel = g[rows, top_idx]                                  # [B, 2]
    gsel = gsel / (gsel.sum(axis=1, keepdims=True) + EPS)
    return gsel.astype(np.float32), top_idx.astype(np.int32)


def _routing_plan(gsel, top_idx):
    """Pick the lo expert set, slot permutation, and per-core sample order."""
    zero = gsel.sum(axis=1) == 0
    pair_mask = np.zeros(B, np.int64)
    for j in range(TOP_K):
        pair_mask |= np.int64(1) << top_idx[:, j].astype(np.int64)
    pair_mask[zero] = 0  # zero-gate rows can claim any slots
    import itertools
    best, best_cnt = None, -1
    for sub in itertools.combinations(range(NUM_EXPERTS), NLO_E):
        msk = np.int64(sum(1 << e for e in sub))
        cnt = int(((pair_mask & ~msk) == 0).sum())
        if cnt > best_cnt:
            best, best_cnt = sub, cnt
    lo_set = list(best)
    hi_set = [e for e in range(NUM_EXPERTS) if e not in lo_set]
    perm = np.empty(NUM_EXPERTS, np.int64)     # expert -> slot
    for slot, e in enumerate(lo_set + hi_set):
        perm[e] = slot

    slot_idx = perm[top_idx]                   # [B, 2]
    slot_idx[zero] = [0, 1]
    low = slot_idx.max(axis=1) < NLO_E

    low_ids = np.where(low)[0]
    high_ids = np.where(~low)[0]
    full_low = len(low_ids) >= NLOW * N_CORES
    order = np.empty((N_CORES, S), np.int64)
    if full_low:
        rest = np.concatenate([low_ids[NLOW * N_CORES:], high_ids])
        for c in range(N_CORES):
            order[c, :NLOW] = low_ids[c * NLOW:(c + 1) * NLOW]
            order[c, NLOW:] = rest[c * (S - NLOW):(c + 1) * (S - NLOW)]
    else:  # fallback: no lo guarantee; program must use full_lowmax
        allb = np.arange(B)
        for c in range(N_CORES):
            order[c] = allb[c * S:(c + 1) * S]
    return perm, slot_idx, order, full_low


def _prep_inputs(cycle_curve_data, logits, moe_masks, W, b):
    gsel, top_idx = _gates_np(logits, moe_masks)
    perm, slot_idx, order, full_low = _routing_plan(gsel, top_idx)

    xf = cycle_curve_data.reshape(B, L, FEAT).astype(np.float32, copy=False)
    # gate-prescaled augmented x: xs[b, j, l, f], f in [0, 901)
    xs = np.empty((B, 2, L, FEAT_AUG), np.float32)
    xs[:, 0, :, :FEAT] = xf * gsel[:, 0, None, None]
    xs[:, 1, :, :FEAT] = xf * gsel[:, 1, None, None]
    xs[:, 0, :, FEAT] = gsel[:, 0, None]
    xs[:, 1, :, FEAT] = gsel[:, 1, None]

    # full[b, p, j, k, l]; k<7 from rows k*128+p, k=7 remainder replicas
    full = np.zeros((B, 128, 2, 8, L), BF16)
    main = xs[:, :, :, :NKF * 128].reshape(B, 2, L, NKF, 128)
    full[:, :, :, :NKF, :] = main.transpose(0, 4, 1, 3, 2).astype(BF16)
    remT = xs[:, :, :, NKF * 128:].transpose(0, 3, 1, 2).astype(BF16)
    for i in range(4):
        full[:, 32 * i:32 * i + REM, :, NKF, :] = remT

    # W with permuted expert slots
    w_aug = np.zeros((NUM_EXPERTS, FEAT_AUG, D_MODEL), np.float32)
    w_aug[perm, :FEAT, :] = W.astype(np.float32)
    w_aug[perm, FEAT, :] = b.astype(np.float32)
    wt = np.zeros((8, 128, NUM_EXPERTS * D_MODEL), BF16)
    wm = w_aug[:, :NKF * 128, :].reshape(NUM_EXPERTS, NKF, 128, D_MODEL)
    wt[:NKF] = (wm.transpose(1, 2, 0, 3)
                .reshape(NKF, 128, NUM_EXPERTS * D_MODEL).astype(BF16))
    wr = w_aug[:, NKF * 128:, :].transpose(1, 0, 2).reshape(
        REM, NUM_EXPERTS * D_MODEL)
    for i in range(4):
        wt[NKF, 32 * i:32 * i + REM, :] = wr.astype(BF16)

    in_maps = []
    for c in range(N_CORES):
        ids = order[c]
        sel = full[ids]                              # [S, 128, 2, 8, L]
        xph1 = np.ascontiguousarray(
            sel[:NP1].transpose(3, 1, 0, 2, 4)       # [k, p, s, j, l]
        ).reshape(8, 128, XCOLS)
        x2 = np.ascontiguousarray(sel[NP1:]).reshape(S - NP1, 128, 2 * 8 * L)
        widx = (slot_idx[ids].reshape(1, 2 * S) * D_MODEL).astype(np.int32)
        in_maps.append({"xph1": xph1, "x2": x2, "w": wt, "widx": widx})
    return in_maps, order, full_low


def kernel(cycle_curve_data, logits, moe_masks, W, b):
    in_maps, order, full_low = _prep_inputs(
        cycle_curve_data, logits, moe_masks, W, b)

    key = "nc" if full_low else "nc_full"
    if key not in _CACHE:
        _CACHE[key] = _build_nc(full_lowmax=not full_low)
    nc = _CACHE[key]

    trace = bool(int(os.environ.get("KERNEL_PROFILE", "0")))
    res = run_bass_kernel_spmd(
        nc, in_maps, core_ids=list(range(N_CORES)), trace=trace
    )
    _CACHE["last_results"] = res

    out = np.empty((B, L, D_MODEL), ml_dtypes.bfloat16)
    for c in range(N_CORES):
        out[order[c]] = res.results[c]["y"]
    return out


# revision 20
# speedup vs baseline: 1.1997x; 1.0174x over previous
"""Trainium2 Bass kernel for BatteryMoEFlattenIntraCycleMoELayer.

Computation (reference):
    gates = renorm(top2(softmax(logits) * mask))          # [B, E]
    x = cycle_curve_data.reshape(B, L, 900)
    out[b] = sum_e gates[b,e] * (x[b] @ W[e] + b[e])      # -> bf16 [B, L, 512]

Strategy (bf16, gate-prescaled x, 7 full K-chunks + row-tiled remainder):
  - Host computes gates/top-2; x is augmented with a bias row (K=901)
    and prescaled by each selected gate -> two copies per sample.
    K = 7 full chunks of 128 + a 5-row remainder (feats 896..899 +
    bias).  Remainder rows are replicated at partitions {0,32,64,96}
    so remainder matmuls for 4 samples run CONCURRENTLY in the 4 PE
    row-groups (tile_position), cutting the old zero-padded chunk-7
    cost ~4x.  (W offsets must stay tile-direct with min_val=0:
    shifting them by a column base broke row-groups 1/3 on HW.)
  - Expert slots are permuted so the 4 most-used experts occupy the
    lo W columns; every core's first 16 samples are chosen (globally,
    via sample permutation) to route only to lo slots, enforced via
    RuntimeValue bounds, so the phase-1 critical stream is
    widx + 8 x [xph1[k] (512 KB) + w_lo[k] (512 KB)] on the Sync
    HWDGE ring -- chunk arrival ~3.6 us vs 3.45 us PE consumption.
    W hi columns stream behind phase 1.
  - Head: ~12 junk matmuls on a zeroed tile warm the PE (HAM 8/8)
    while the first DMAs land.  Phase-2 per-sample x rides the Scalar
    (ACT) HWDGE ring behind a FIFO gate (a dummy read of w_sb[7])
    that releases only after the phase-1 stream has fully landed;
    phase-2 x tiles also share a 9-buffer pool with the phase-1
    tiles, so their DMA self-throttles to the compute rate.
  - Phase 1: samples 0-7 k-outer (8 PSUM banks).  Phase 2: quads of
    sample-major full chunks + one concurrent remainder batch pair;
    combines on DVE (tensor_scalar_add, psum f32 -> sbuf bf16), y on
    the Sync ring; in the last quad the final sample combines/stores
    first to shorten the tail.
  - Shard B across 8 cores (64 samples each, host-permuted; output
    inverse-permuted).
"""

import os
import sys

for _p in ("/opt/trn_rl_repo", "/root/.axon_site/_ro/trn_rl_repo"):
    if os.path.isdir(_p) and _p not in sys.path:
        sys.path.insert(0, _p)

import numpy as np
import ml_dtypes

import concourse.bass as bass
import concourse.mybir as mybir
import concourse.tile as tile
from concourse import bacc
from concourse.bass_utils import run_bass_kernel_spmd
from concourse.bass_values import RuntimeValue

B, L, CURVE_LEN = 512, 128, 300
FEAT = 3 * CURVE_LEN          # 900
FEAT_AUG = FEAT + 1           # 901 (bias row)
NKF = 7                       # full 128-row K chunks (rows 0..895)
REM = FEAT_AUG - NKF * 128    # 5 remainder rows (896..899 + bias)
D_MODEL = 512
NUM_EXPERTS = 8
TOP_K = 2
EPS = 1e-9
N_CORES = 8
S = B // N_CORES              # 64 samples per core
NP1 = 8                       # phase-1 k-outer group size (PSUM banks)
NLOW = 16                     # per-core samples guaranteed lo-routed
NLO_E = 4                     # experts in the lo slot group
XCOLS = 2 * NP1 * L           # 2048: phase-1 x columns per chunk
WBASE = 0                     # widx offsets are direct W columns
LO_COLS = NLO_E * D_MODEL     # 2048
WCOLS = NUM_EXPERTS * D_MODEL # 4096
LOWMAX = (NLO_E - 1) * D_MODEL
WMAX = (NUM_EXPERTS - 1) * D_MODEL
NJUNK = 12                    # PE-warmup matmuls

BF16 = ml_dtypes.bfloat16

_CACHE = {}


def _build_nc(full_lowmax=False):
    """Build the SPMD Bass program (routing carried as data)."""
    nc = bacc.Bacc(trn_type="TRN2")
    f32 = mybir.dt.float32
    bf16 = mybir.dt.bfloat16
    i32 = mybir.dt.int32

    lowmax = WMAX if full_lowmax else LOWMAX

    # phase-1 x, k-major: col = (s*2 + j)*128 + l for samples 0..7
    xph1_h = nc.declare_dram_parameter("xph1", [8, 128, XCOLS], bf16,
                                       isOutput=False)
    # w per k-chunk: [k, part, slot*512]; chunk 7 = remainder rows
    # replicated at partitions {0,32,64,96}
    w_h = nc.declare_dram_parameter("w", [8, 128, WCOLS], bf16,
                                    isOutput=False)
    # phase-2 x, sample-major: col = (j*8 + k)*128 + l  (k=7 -> remainder)
    x2_h = nc.declare_dram_parameter("x2", [S - NP1, 128, 2 * 8 * L], bf16,
                                     isOutput=False)
    widx_h = nc.declare_dram_parameter("widx", [1, 2 * S], i32, isOutput=False)
    y_h = nc.declare_dram_parameter("y", [S, L, D_MODEL], bf16, isOutput=True)

    with tile.TileContext(nc) as tc:
        with (
            tc.tile_pool(name="cpool", bufs=1) as cpool,
            tc.tile_pool(name="xpool", bufs=9) as xpool,
            tc.tile_pool(name="opool", bufs=10) as opool,
            tc.tile_pool(name="pspool", bufs=8, space="PSUM") as pspool,
        ):
            # ---- head: widx + junk-warmup ----
            widx_sb = cpool.tile([1, 2 * S], i32)
            nc.sync.dma_start(out=widx_sb[:, :], in_=widx_h[:, :])

            junk = xpool.tile([128, 2 * 8 * L], bf16, tag="x", name="junk")
            nc.vector.memset(junk[:, 0:640], 0.0)
            ps_junk = pspool.tile([128, D_MODEL], f32, tag="ps",
                                  name="ps_junk")
            for _ in range(NJUNK):
                nc.tensor.matmul(ps_junk[:, :], junk[:, 0:128],
                                 junk[:, 128:640], start=True, stop=True)

            # ---- critical stream, paced across BOTH HWDGE rings ----
            # w_lo[k] rides Sync; xph1[k] rides Scalar(ACT), but each
            # xph1 trigger (k>=1) sits behind a dummy ACT read of
            # w_sb[k-1], so the ACT ring never queues more than one
            # transfer and cannot starve the W stream (packet-level
            # round-robin shares SDMA engines between the rings).
            pace_sb = cpool.tile([1, 16], bf16, name="pace_sb")
            xph1_sb = []
            w_sb = []
            for k in range(8):
                if k >= 1:
                    nc.scalar.copy(pace_sb[0:1, :], w_sb[k - 1][0:1, 0:16])
                xt = xpool.tile([128, XCOLS], bf16, tag="x",
                                name=f"xph1_{k}")
                nc.scalar.dma_start(out=xt[:, :], in_=xph1_h[k, :, :])
                xph1_sb.append(xt)
                wt = cpool.tile([128, WCOLS], bf16, name=f"w_sb_{k}")
                nc.sync.dma_start(out=wt[:, 0:LO_COLS],
                                  in_=w_h[k, :, 0:LO_COLS])
                w_sb.append(wt)

            # ring of PE registers for per-sample W-slot offsets
            NRING = 16
            wregs = [nc.tensor.alloc_register(f"widx_reg{i}")
                     for i in range(NRING)]

            rv_of = {}

            def load_rv8(s0, maxv):
                # 8 registers <- widx for samples s0..s0+3 in one load;
                # consecutive batches land in alternating ring halves
                regs = [wregs[(2 * s0 + j) % NRING] for j in range(8)]
                nc.tensor.reg_load(regs, widx_sb[0:1, 2 * s0: 2 * s0 + 8])
                for j in range(4):
                    if s0 + j < S:
                        rv_of[s0 + j] = (
                            RuntimeValue(val=regs[2 * j], min_val=WBASE,
                                         max_val=maxv),
                            RuntimeValue(val=regs[2 * j + 1], min_val=WBASE,
                                         max_val=maxv),
                        )

            def load_rv16(s0, maxv):
                load_rv8(s0, maxv)
                load_rv8(s0 + 4, maxv)

            def mm_full(ps, lhs, k, rv, start):
                nc.tensor.matmul(
                    ps[:, :], lhs,
                    w_sb[k][:, bass.ds(rv, D_MODEL)],
                    start=start, stop=False,
                )

            def mm_rem(ps, lhs_tile, col, i, rv, stop):
                # remainder rows at partitions 32i..32i+4; 4 distinct
                # row-groups run concurrently on the PE
                nc.tensor.matmul(
                    ps[:, :],
                    lhs_tile[32 * i: 32 * i + REM, col: col + L],
                    w_sb[7][32 * i: 32 * i + REM, bass.ds(rv, D_MODEL)],
                    start=False, stop=stop,
                    tile_position=(32 * i, 0),
                )

            def combine(s, ps):
                o_sb = opool.tile([128, D_MODEL], bf16, tag="o", name=f"o_{s}")
                if s == S - 1:
                    # final sample: halve combine/store so the first y
                    # bytes leave ~0.35 us earlier (shorter kernel tail)
                    for h0 in (0, D_MODEL // 2):
                        h1 = h0 + D_MODEL // 2
                        nc.vector.tensor_scalar_add(o_sb[:, h0:h1],
                                                    ps[:, h0:h1], 0.0)
                        nc.sync.dma_start(out=y_h[s, :, h0:h1],
                                          in_=o_sb[:, h0:h1])
                else:
                    nc.vector.tensor_scalar_add(o_sb[:, :], ps[:, :], 0.0)
                    nc.sync.dma_start(out=y_h[s, :, :], in_=o_sb[:, :])

            # ---- phase 1: samples 0..7, k-outer ----
            load_rv16(0, lowmax)
            ps1 = {s: pspool.tile([128, D_MODEL], f32, tag="ps",
                                  name=f"ps_{s}") for s in range(NP1)}
            for k in range(NKF):
                for s in range(NP1):
                    rvA, rvB = rv_of[s]
                    mm_full(ps1[s],
                            xph1_sb[k][:, (2 * s) * L:(2 * s + 1) * L],
                            k, rvA, start=(k == 0))
                    mm_full(ps1[s],
                            xph1_sb[k][:, (2 * s + 1) * L:(2 * s + 2) * L],
                            k, rvB, start=False)

            # w hi columns stream behind the phase-1 critical path
            for k in range(8):
                nc.sync.dma_start(out=w_sb[k][:, LO_COLS:WCOLS],
                                  in_=w_h[k, :, LO_COLS:WCOLS])

            # phase-1 remainder: concurrent 4-slot batches, then combine
            for g in range(2):
                for j in range(2):
                    for i in range(4):
                        s = 4 * g + i
                        rv = rv_of[s][j]
                        mm_rem(ps1[s], xph1_sb[7], (2 * s + j) * L, i, rv,
                               stop=(j == 1))
                for i in range(4):
                    s = 4 * g + i
                    combine(s, ps1[s])

            # ---- phase 2: samples 8..63, quad-major ----
            # FIFO gate on the ACT ring: the first x2 trigger sits behind
            # this read of h_sb[7], so phase-2 x DMA can't steal HBM
            # bandwidth from the phase-1 critical stream.
            gate_sb = cpool.tile([1, 16], bf16, name="gate_sb")
            nc.scalar.copy(gate_sb[0:1, :], w_sb[7][0:1, 0:16])

            # rem batching in hexads (6 samples) + a final duo: fewer
            # full<->rem PE config transitions per sample than quads,
            # while group + successor PSUM banks stay within 8.
            group_end = {NP1 + 6 * h + 5 for h in range(9)} | {S - 1}
            group_of = []
            g0 = NP1
            for s in range(NP1, S):
                group_of.append(g0)
                if s in group_end:
                    g0 = s + 1

            x2_sb = {}
            psq = {}
            for s in range(NP1, S):
                xt = xpool.tile([128, 2 * 8 * L], bf16, tag="x",
                                name=f"x2_{s}")
                nc.scalar.dma_start(out=xt[:, :], in_=x2_h[s - NP1, :, :])
                x2_sb[s] = xt

                if s % 4 == 0:
                    load_rv8(s, lowmax if s + 4 <= NLOW else WMAX)
                rvA, rvB = rv_of[s]

                ps = pspool.tile([128, D_MODEL], f32, tag="ps",
                                 name=f"ps2_{s}")
                psq[s] = ps
                for k in range(NKF):
                    mm_full(ps, xt[:, k * L:(k + 1) * L], k, rvA,
                            start=(k == 0))
                    mm_full(ps, xt[:, (8 + k) * L:(9 + k) * L], k, rvB,
                            start=False)

                if s in group_end:
                    q0 = group_of[s - NP1]
                    n = s - q0 + 1
                    # (sample, j) slots in batches of 4 across row-groups;
                    # each bank's j=1 rem MM is pc-later than its j=0 one,
                    # so stop rides the j=1 MM.
                    slots = [(q0 + i, j) for j in range(2) for i in range(n)]
                    bs = 4 if n >= 4 else n  # a batch must not repeat a bank
                    for bi in range(0, 2 * n, bs):
                        for rg, (s2, j) in enumerate(slots[bi:bi + bs]):
                            rv = rv_of[s2][j]
                            mm_rem(psq[s2], x2_sb[s2], (j * 8 + 7) * L, rg,
                                   rv, stop=(j == 1))
                    # last group: combine/store the final sample FIRST so
                    # its y DMA isn't serialized behind other combines
                    idxs = range(n - 1, -1, -1) if s == S - 1 else range(n)
                    for i in idxs:
                        s2 = q0 + i
                        combine(s2, psq[s2])
                        del x2_sb[s2], psq[s2]

    nc.finalize()
    return nc


def _gates_np(logits, moe_masks):
    """Mirror reference _gates in numpy (fp32)."""
    lg = logits.astype(np.float32)
    m = lg.max(axis=1, keepdims=True)
    e = np.exp(lg - m)
    g = e / e.sum(axis=1, keepdims=True)
    g = g * (moe_masks == 1).astype(np.float32)
    # top-2, ties -> lower index first (matches jax.lax.top_k)
    top_idx = np.argsort(-g, axis=1, kind="stable")[:, :TOP_K]
    rows = np.arange(g.shape[0])[:, None]
    gsel = g[rows, top_idx]                                  # [B, 2]
    gsel = gsel / (gsel.sum(axis=1, keepdims=True) + EPS)
    return gsel.astype(np.float32), top_idx.astype(np.int32)


def _routing_plan(gsel, top_idx):
    """Pick the lo expert set, slot permutation, and per-core sample order."""
    zero = gsel.sum(axis=1) == 0
    pair_mask = np.zeros(B, np.int64)
    for j in range(TOP_K):
        pair_mask |= np.int64(1) << top_idx[:, j].astype(np.int64)
    pair_mask[zero] = 0  # zero-gate rows can claim any slots
    import itertools
    best, best_cnt = None, -1
    for sub in itertools.combinations(range(NUM_EXPERTS), NLO_E):
        msk = np.int64(sum(1 << e for e in sub))
        cnt = int(((pair_mask & ~msk) == 0).sum())
        if cnt > best_cnt:
            best, best_cnt = sub, cnt
    lo_set = list(best)
    hi_set = [e for e in range(NUM_EXPERTS) if e not in lo_set]
    perm = np.empty(NUM_EXPERTS, np.int64)     # expert -> slot
    for slot, e in enumerate(lo_set + hi_set):
        perm[e] = slot

    slot_idx = perm[top_idx]                   # [B, 2]
    slot_idx[zero] = [0, 1]
    low = slot_idx.max(axis=1) < NLO_E

    low_ids = np.where(low)[0]
    high_ids = np.where(~low)[0]
    full_low = len(low_ids) >= NLOW * N_CORES
    order = np.empty((N_CORES, S), np.int64)
    if full_low:
        rest = np.concatenate([low_ids[NLOW * N_CORES:], high_ids])
        for c in range(N_CORES):
            order[c, :NLOW] = low_ids[c * NLOW:(c + 1) * NLOW]
            order[c, NLOW:] = rest[c * (S - NLOW):(c + 1) * (S - NLOW)]
    else:  # fallback: no lo guarantee; program must use full_lowmax
        allb = np.arange(B)
        for c in range(N_CORES):
            order[c] = allb[c * S:(c + 1) * S]
    return perm, slot_idx, order, full_low


def _prep_inputs(cycle_curve_data, logits, moe_masks, W, b):
    gsel, top_idx = _gates_np(logits, moe_masks)
    perm, slot_idx, order, full_low = _routing_plan(gsel, top_idx)

    xf = cycle_curve_data.reshape(B, L, FEAT).astype(np.float32, copy=False)
    # gate-prescaled augmented x: xs[b, j, l, f], f in [0, 901)
    xs = np.empty((B, 2, L, FEAT_AUG), np.float32)
    xs[:, 0, :, :FEAT] = xf * gsel[:, 0, None, None]
    xs[:, 1, :, :FEAT] = xf * gsel[:, 1, None, None]
    xs[:, 0, :, FEAT] = gsel[:, 0, None]
    xs[:, 1, :, FEAT] = gsel[:, 1, None]

    # full[b, p, j, k, l]; k<7 from rows k*128+p, k=7 remainder replicas
    full = np.zeros((B, 128, 2, 8, L), BF16)
    main = xs[:, :, :, :NKF * 128].reshape(B, 2, L, NKF, 128)
    full[:, :, :, :NKF, :] = main.transpose(0, 4, 1, 3, 2).astype(BF16)
    remT = xs[:, :, :, NKF * 128:].transpose(0, 3, 1, 2).astype(BF16)
    for i in range(4):
        full[:, 32 * i:32 * i + REM, :, NKF, :] = remT

    # W with permuted expert slots
    w_aug = np.zeros((NUM_EXPERTS, FEAT_AUG, D_MODEL), np.float32)
    w_aug[perm, :FEAT, :] = W.astype(np.float32)
    w_aug[perm, FEAT, :] = b.astype(np.float32)
    wt = np.zeros((8, 128, NUM_EXPERTS * D_MODEL), BF16)
    wm = w_aug[:, :NKF * 128, :].reshape(NUM_EXPERTS, NKF, 128, D_MODEL)
    wt[:NKF] = (wm.transpose(1, 2, 0, 3)
                .reshape(NKF, 128, NUM_EXPERTS * D_MODEL).astype(BF16))
    wr = w_aug[:, NKF * 128:, :].transpose(1, 0, 2).reshape(
        REM, NUM_EXPERTS * D_MODEL)
    for i in range(4):
        wt[NKF, 32 * i:32 * i + REM, :] = wr.astype(BF16)

    in_maps = []
    for c in range(N_CORES):
        ids = order[c]
        sel = full[ids]                              # [S, 128, 2, 8, L]
        xph1 = np.ascontiguousarray(
            sel[:NP1].transpose(3, 1, 0, 2, 4)       # [k, p, s, j, l]
        ).reshape(8, 128, XCOLS)
        x2 = np.ascontiguousarray(sel[NP1:]).reshape(S - NP1, 128, 2 * 8 * L)
        widx = (slot_idx[ids].reshape(1, 2 * S) * D_MODEL).astype(np.int32)
        in_maps.append({"xph1": xph1, "x2": x2, "w": wt, "widx": widx})
    return in_maps, order, full_low


def kernel(cycle_curve_data, logits, moe_masks, W, b):
    in_maps, order, full_low = _prep_inputs(
        cycle_curve_data, logits, moe_masks, W, b)

    key = "nc" if full_low else "nc_full"
    if key not in _CACHE:
        _CACHE[key] = _build_nc(full_lowmax=not full_low)
    nc = _CACHE[key]

    trace = bool(int(os.environ.get("KERNEL_PROFILE", "0")))
    res = run_bass_kernel_spmd(
        nc, in_maps, core_ids=list(range(N_CORES)), trace=trace
    )
    _CACHE["last_results"] = res

    out = np.empty((B, L, D_MODEL), ml_dtypes.bfloat16)
    for c in range(N_CORES):
        out[order[c]] = res.results[c]["y"]
    return out
